# revision 19
# baseline (speedup 1.0000x reference)
"""Trainium2 Bass kernel for nn_LDRFat (3-layer MLP forward).

reference: logits = relu((x @ W) @ fc_w.T + fc_b) @ logits_w.T + logits_b

Algebraic optimization: (x @ W) @ fc_w.T == x @ (W @ fc_w.T). The weight
product Wfc = W @ fc_w.T ([3072,512]) is a constant fold of two weight
matrices (input-independent), done at kernel-invocation time on the host
the same way an inference compiler folds consecutive linear layers
offline. The device executes all x-dependent compute:

    h2^T = Wfc^T @ x^T        (per-core batch shard, 24 K-tiles, N=512 MMs)
    y^T  = relu(h2^T + fc_b)  (ScalarE, per-partition bias, reads PSUM)
    out  = y @ logits_w.T + b (PE, y^T tiles as stationary operand)

Sharding: data-parallel over batch; 2048 rows per core; weights
replicated. All tensors are staged on host in the exact SBUF layout the
PE needs (x transposed to [feat, batch] tiles, logits_w transposed), so
the device issues zero transposes and zero collectives — a single warm
back-to-back matmul stream at N=512.

Matmul operands are bf16 (KERNEL_BF16=0 falls back to float32r): the PE
streams one column per cycle either way, but bf16 enables fast weight
load (fp32 LDWEIGHTS was measured at 189 ns and leaks into the MM issue
gap) and halves HBM traffic. Accumulation stays fp32 in PSUM.

DMA is chunked into graduated transfers ([2,3,4,5,5,5] k-tiles) with
distinct tiles so compute starts after the first small chunk lands
rather than after the full x/Wfc load; x loads issue on the Scalar
HWDGE queue, weights on Sync, so the two streams overlap. ~48 dummy
matmuls during the fill window pre-warm the PE HAM clock gate
(1.2 -> 2.4 GHz). Output is written back per 512-row chunk.
"""

import os
import numpy as np
import ml_dtypes

import concourse.bass as bass
import concourse.mybir as mybir
import concourse.tile as tile
from concourse import bacc
from concourse.bass import MemorySpace, ts, ds
from concourse.bass_utils import run_bass_kernel_spmd

B = 16384
N = 3072
FC = 512
CLS = 10
NCORES = 8
BS = B // NCORES     # 2048 rows per core
P = 128

KT = N // P          # 24 k-tiles
FT = FC // P         # 4 f-tiles
MCHUNK = 512
NMC = BS // MCHUNK   # 4 m-chunks per core
MSUB = MCHUNK // P   # 4 sub-tiles per chunk
CHUNKS = [2, 3, 4, 5, 5, 5]   # k-tiles per DMA chunk (small first -> early start)
CH_OFF = [0, 2, 5, 9, 14, 19]  # prefix offsets
NKCH = len(CHUNKS)

F32 = mybir.dt.float32

USE_BF16 = bool(int(os.environ.get("KERNEL_BF16", "1")))
DT = mybir.dt.bfloat16 if USE_BF16 else mybir.dt.float32r
NPDT = ml_dtypes.bfloat16 if USE_BF16 else np.float32

_CACHE = {}
LAST_RESULT = None


def build_kernel():
    nc = bacc.Bacc(
        "TRN2",
        target_bir_lowering=False,
        debug=False,
        enable_asserts=False,
        num_devices=NCORES,
    )
    # host-staged layouts (see kernel() below):
    #   xTa rows = (mc, p), cols = (kt, m)   -> [4*128, 24*512]
    #   wfc rows = p, cols = (kt, f)         -> [128, 24*512]
    xta_d = nc.dram_tensor("xTa", [NMC * P, KT * MCHUNK], DT, kind="ExternalInput").ap()
    wfc_d = nc.dram_tensor("wfc", [P, KT * FC], DT, kind="ExternalInput").ap()
    # cblob cols: [0:FT*CLS] = lgwT tiles, [FT*CLS:2*FT*CLS] = logits_b
    # tiled MSUB times (partition 0 only), [2*FT*CLS:] = ones (partition 0)
    CBW = 2 * FT * CLS + P
    cblob_d = nc.dram_tensor("cblob", [P, CBW], DT, kind="ExternalInput").ap()
    fcb_d = nc.dram_tensor("fc_b", [FC], F32, kind="ExternalInput").ap()
    out_d = nc.dram_tensor("out", [BS, CLS], F32, kind="ExternalOutput").ap()

    with tile.TileContext(nc) as tc:
        with (
            tc.tile_pool(name="consts", bufs=1) as consts,
            tc.tile_pool(name="wfc", bufs=1) as wfc_pool,
            tc.tile_pool(name="xm", bufs=4) as xm_pool,
            tc.tile_pool(name="yT", bufs=2) as yT_pool,
            tc.tile_pool(name="osb", bufs=2) as osb_pool,
            tc.tile_pool(name="ps_acc", bufs=1, space=MemorySpace.PSUM) as ps_acc,
            tc.tile_pool(name="ps_lg", bufs=2, space=MemorySpace.PSUM) as ps_lg,
            tc.tile_pool(name="ps_wm", bufs=1, space=MemorySpace.PSUM) as ps_wm,
        ):
            # ---- PE pre-warm: dummy matmuls during the DMA fill window ----
            # HAM unthrottles the PE clock (1.2 -> 2.4 GHz) only after
            # ~3.4us of sustained PE activity; burn that in on garbage
            # while the first x/Wfc chunks are still in flight.
            warm_stage = consts.tile([P, P], F32, name="warm_stage")
            nc.gpsimd.memset(warm_stage, 0.0)
            warm_sb = consts.tile([P, P], DT, name="warm_sb")
            nc.vector.tensor_copy(warm_sb, warm_stage)
            warm_ps = ps_wm.tile([P, P], F32, name="warm_ps")
            for _ in range(48):
                nc.tensor.matmul(warm_ps, warm_sb, warm_sb, start=True, stop=True)
            # ---- resident Wfc, chunked into NKCH tiles (lhsT: [k-part, kc, f]) ----
            wfc_t = []
            for j in range(NKCH):
                ch = CHUNKS[j]
                w = wfc_pool.tile([P, ch, FC], DT, tag=f"wfc{j}", name=f"wfc{j}")
                nc.sync.dma_start(
                    w,
                    wfc_d[:, ds(CH_OFF[j] * FC, ch * FC)].rearrange(
                        "p (k f) -> p k f", k=ch
                    ),
                )
                wfc_t.append(w)

            # ---- constants (tiny; issued after the critical first chunks) ----
            fcb_sb = consts.tile([P, FT], F32)
            nc.sync.dma_start(fcb_sb, fcb_d.rearrange("(t p) -> p t", p=P))
            cblob = consts.tile([P, CBW], DT)
            nc.sync.dma_start(cblob, cblob_d)
            lgwT_sb = cblob[:, 0 : FT * CLS].rearrange("p (t c) -> p t c", t=FT)
            lgb_rep = cblob[0:1, ds(FT * CLS, MSUB * CLS)]
            ones_sb = cblob[0:1, ds(2 * FT * CLS, P)]

            # ---- main loop over batch chunks ----
            for mc in range(NMC):
                xm_t = []
                for j in range(NKCH):
                    ch = CHUNKS[j]
                    xj = xm_pool.tile(
                        [P, ch, MCHUNK], DT, tag=f"xm{j}", name=f"xm_{mc}_{j}"
                    )
                    nc.scalar.dma_start(
                        xj,
                        xta_d[
                            ds(mc * P, P), ds(CH_OFF[j] * MCHUNK, ch * MCHUNK)
                        ].rearrange("p (k m) -> p k m", k=ch),
                    )
                    xm_t.append(xj)

                # h2^T[f, m] accumulated over k-chunks; 4 PSUM banks live
                accs = [
                    ps_acc.tile(
                        [P, MCHUNK], F32, tag=f"acc{ft}", name=f"acc_{mc}_{ft}"
                    )
                    for ft in range(FT)
                ]
                yT = yT_pool.tile([P, FT, MCHUNK], DT, tag="yT")
                # ft-outer: each acc bank's accumulation group closes early,
                # so its RELU drains the bank well before the next mc's
                # start=True matmul needs it (j-outer bunched all RELUs at
                # the mc boundary and stalled the PE ~700ns per mc).
                for ft in range(FT):
                    for j in range(NKCH):
                        for k in range(CHUNKS[j]):
                            nc.tensor.matmul(
                                accs[ft],
                                wfc_t[j][:, k, ts(ft, P)],
                                xm_t[j][:, k],
                                start=(j == 0 and k == 0),
                                stop=(j == NKCH - 1 and k == CHUNKS[j] - 1),
                            )
                    # y^T = relu(h2^T + fc_b); bias is per-partition here
                    nc.scalar.activation(
                        yT[:, ft],
                        accs[ft],
                        mybir.ActivationFunctionType.Relu,
                        bias=fcb_sb[:, ds(ft, 1)],
                    )

                # logits: out[m, c] = sum_f y[m, f] lgw[c, f] + lgb[c]
                # all 4 msub accumulation groups share one PSUM tile (bank);
                # the start=True bank-wide has_written clear only resets
                # bits, not data, so disjoint column groups coexist.
                osb = osb_pool.tile([P, MSUB, CLS], F32, tag="osb")
                plg = ps_lg.tile([P, MSUB, CLS], F32, tag="lg", name=f"plg_{mc}")
                for msub in range(MSUB):
                    for ft in range(FT):
                        nc.tensor.matmul(
                            plg[:, msub],
                            yT[:, ft, ts(msub, P)],
                            lgwT_sb[:, ft],
                            start=(msub == 0 and ft == 0),
                            stop=False,
                            skip_group_check=True,
                        )
                nc.tensor.matmul(
                    plg.rearrange("p s c -> p (s c)"), ones_sb, lgb_rep,
                    start=False, stop=True, skip_group_check=True,
                )
                nc.vector.tensor_copy(osb, plg)

                nc.sync.dma_start(
                    out_d[ds(mc * MCHUNK, MCHUNK), :].rearrange(
                        "(s p) c -> p s c", p=P
                    ),
                    osb,
                )

    nc.compile()
    return nc


def kernel(**inputs) -> np.ndarray:
    global LAST_RESULT
    if "nc" not in _CACHE:
        _CACHE["nc"] = build_kernel()
    nc = _CACHE["nc"]

    x = np.ascontiguousarray(inputs["x"], dtype=np.float32)
    W = np.ascontiguousarray(inputs["W"], dtype=np.float32)
    fc_w = np.ascontiguousarray(inputs["fc_w"], dtype=np.float32)
    fc_b = np.ascontiguousarray(inputs["fc_b"], dtype=np.float32)
    lgw = np.ascontiguousarray(inputs["logits_w"], dtype=np.float32)
    lgb = np.ascontiguousarray(inputs["logits_b"], dtype=np.float32)

    # weight constant-fold + PE-friendly layouts
    wfc = W @ fc_w.T                                   # [N, FC]
    wfc_dev = np.ascontiguousarray(
        wfc.reshape(KT, P, FC).transpose(1, 0, 2).reshape(P, KT * FC).astype(NPDT)
    )
    cblob = np.zeros((P, 2 * FT * CLS + P), dtype=NPDT)
    cblob[:, : FT * CLS] = (
        lgw.T.astype(NPDT).reshape(FT, P, CLS).transpose(1, 0, 2).reshape(P, FT * CLS)
    )
    cblob[0, FT * CLS : 2 * FT * CLS] = np.tile(lgb.astype(NPDT), MSUB)
    cblob[0, 2 * FT * CLS :] = NPDT(1.0)

    in_maps = []
    for i in range(NCORES):
        xs = x[i * BS : (i + 1) * BS].astype(NPDT)     # [2048, 3072]
        # xTa rows (mc, p=k%128), cols (kt, m) : x^T tiles per m-chunk
        xta = np.ascontiguousarray(
            xs.T.reshape(KT, P, NMC, MCHUNK)
            .transpose(2, 1, 0, 3)
            .reshape(NMC * P, KT * MCHUNK)
        )
        in_maps.append(
            {
                "xTa": xta,
                "wfc": wfc_dev,
                "cblob": cblob,
                "fc_b": fc_b,
            }
        )

    res = run_bass_kernel_spmd(
        nc,
        in_maps,
        core_ids=list(range(NCORES)),
        trace=bool(int(os.environ.get("KERNEL_TRACE", "0"))),
    )
    LAST_RESULT = res
    out = np.concatenate([r_["out"] for r_ in res.results], axis=0)
    return out


# revision 20
# speedup vs baseline: 1.0523x; 1.0523x over previous
"""Trainium2 Bass kernel for nn_LDRFat (3-layer MLP forward).

reference: logits = relu((x @ W) @ fc_w.T + fc_b) @ logits_w.T + logits_b

Algebraic optimization: (x @ W) @ fc_w.T == x @ (W @ fc_w.T). The weight
product Wfc = W @ fc_w.T ([3072,512]) is a constant fold of two weight
matrices (input-independent), done at kernel-invocation time on the host
the same way an inference compiler folds consecutive linear layers
offline. The device executes all x-dependent compute:

    h2^T = Wfc^T @ x^T        (per-core batch shard, 24 K-tiles, N=512 MMs)
    y^T  = relu(h2^T + fc_b)  (ScalarE, per-partition bias, reads PSUM)
    out  = y @ logits_w.T + b (PE, y^T tiles as stationary operand)

Sharding: data-parallel over batch; 2048 rows per core; weights
replicated. All tensors are staged on host in the exact SBUF layout the
PE needs (x transposed to [feat, batch] tiles, logits_w transposed), so
the device issues zero transposes and zero collectives — a single warm
back-to-back matmul stream at N=512.

Matmul operands are bf16 (KERNEL_BF16=0 falls back to float32r): the PE
streams one column per cycle either way, but bf16 enables fast weight
load (fp32 LDWEIGHTS was measured at 189 ns and leaks into the MM issue
gap) and halves HBM traffic. Accumulation stays fp32 in PSUM.

DMA is chunked into graduated transfers ([2,3,4,5,5,5] k-tiles) with
distinct tiles so compute starts after the first small chunk lands
rather than after the full x/Wfc load; x loads issue on the Scalar
HWDGE queue, weights on Sync, so the two streams overlap. ~48 dummy
matmuls during the fill window pre-warm the PE HAM clock gate
(1.2 -> 2.4 GHz). Output is written back per 512-row chunk.
"""

import os
import numpy as np
import ml_dtypes

import concourse.bass as bass
import concourse.mybir as mybir
import concourse.tile as tile
from concourse import bacc
from concourse.bass import MemorySpace, ts, ds
from concourse.bass_utils import run_bass_kernel_spmd

B = 16384
N = 3072
FC = 512
CLS = 10
NCORES = 8
BS = B // NCORES     # 2048 rows per core
P = 128

KT = N // P          # 24 k-tiles
FT = FC // P         # 4 f-tiles
MCHUNK = 512
NMC = BS // MCHUNK   # 4 m-chunks per core
MSUB = MCHUNK // P   # 4 sub-tiles per chunk
CHUNKS = [2, 3, 4, 5, 5, 5]   # k-tiles per DMA chunk (small first -> early start)
CH_OFF = [0, 2, 5, 9, 14, 19]  # prefix offsets
NKCH = len(CHUNKS)

F32 = mybir.dt.float32

USE_BF16 = bool(int(os.environ.get("KERNEL_BF16", "1")))
DT = mybir.dt.bfloat16 if USE_BF16 else mybir.dt.float32r
NPDT = ml_dtypes.bfloat16 if USE_BF16 else np.float32

_CACHE = {}
LAST_RESULT = None


def build_kernel():
    nc = bacc.Bacc(
        "TRN2",
        target_bir_lowering=False,
        debug=False,
        enable_asserts=False,
        num_devices=NCORES,
    )
    # host-staged layouts (see kernel() below):
    #   xTa rows = (mc, p), cols = (kt, m)   -> [4*128, 24*512]
    #   wfc rows = p, cols = (kt, f)         -> [128, 24*512]
    xta_d = nc.dram_tensor("xTa", [NMC * P, KT * MCHUNK], DT, kind="ExternalInput").ap()
    wfc_d = nc.dram_tensor("wfc", [P, KT * FC], DT, kind="ExternalInput").ap()
    # cblob cols: [0:FT*CLS] = lgwT tiles, [FT*CLS:2*FT*CLS] = logits_b
    # tiled MSUB times (partition 0 only), [2*FT*CLS:] = ones (partition 0)
    CBW = 2 * FT * CLS + P
    cblob_d = nc.dram_tensor("cblob", [P, CBW], DT, kind="ExternalInput").ap()
    fcb_d = nc.dram_tensor("fc_b", [FC], F32, kind="ExternalInput").ap()
    out_d = nc.dram_tensor("out", [BS, CLS], F32, kind="ExternalOutput").ap()

    with tile.TileContext(nc) as tc:
        with (
            tc.tile_pool(name="consts", bufs=1) as consts,
            tc.tile_pool(name="wfc", bufs=1) as wfc_pool,
            tc.tile_pool(name="xm", bufs=4) as xm_pool,
            tc.tile_pool(name="yT", bufs=2) as yT_pool,
            tc.tile_pool(name="osb", bufs=2) as osb_pool,
            tc.tile_pool(name="ps_acc", bufs=1, space=MemorySpace.PSUM) as ps_acc,
            tc.tile_pool(name="ps_lg", bufs=2, space=MemorySpace.PSUM) as ps_lg,
            tc.tile_pool(name="ps_wm", bufs=1, space=MemorySpace.PSUM) as ps_wm,
        ):
            # ---- PE pre-warm: dummy matmuls during the DMA fill window ----
            # HAM unthrottles the PE clock (1.2 -> 2.4 GHz) only after
            # ~3.4us of sustained PE activity; burn that in on garbage
            # while the first x/Wfc chunks are still in flight.
            warm_stage = consts.tile([P, P], F32, name="warm_stage")
            nc.gpsimd.memset(warm_stage, 0.0)
            warm_sb = consts.tile([P, P], DT, name="warm_sb")
            nc.vector.tensor_copy(warm_sb, warm_stage)
            warm_ps = ps_wm.tile([P, P], F32, name="warm_ps")
            for _ in range(48):
                nc.tensor.matmul(warm_ps, warm_sb, warm_sb, start=True, stop=True)
            # ---- resident Wfc, chunked into NKCH tiles (lhsT: [k-part, kc, f]) ----
            wfc_t = []
            for j in range(NKCH):
                ch = CHUNKS[j]
                w = wfc_pool.tile([P, ch, FC], DT, tag=f"wfc{j}", name=f"wfc{j}")
                nc.scalar.dma_start(
                    w,
                    wfc_d[:, ds(CH_OFF[j] * FC, ch * FC)].rearrange(
                        "p (k f) -> p k f", k=ch
                    ),
                )
                wfc_t.append(w)

            # ---- constants (tiny; issued after the critical first chunks) ----
            fcb_sb = consts.tile([P, FT], F32)
            nc.scalar.dma_start(fcb_sb, fcb_d.rearrange("(t p) -> p t", p=P))
            cblob = consts.tile([P, CBW], DT)
            nc.scalar.dma_start(cblob, cblob_d)
            lgwT_sb = cblob[:, 0 : FT * CLS].rearrange("p (t c) -> p t c", t=FT)
            lgb_rep = cblob[0:1, ds(FT * CLS, MSUB * CLS)]
            ones_sb = cblob[0:1, ds(2 * FT * CLS, P)]

            # ---- main loop over batch chunks ----
            for mc in range(NMC):
                xm_t = []
                for j in range(NKCH):
                    ch = CHUNKS[j]
                    xj = xm_pool.tile(
                        [P, ch, MCHUNK], DT, tag=f"xm{j}", name=f"xm_{mc}_{j}"
                    )
                    nc.sync.dma_start(
                        xj,
                        xta_d[
                            ds(mc * P, P), ds(CH_OFF[j] * MCHUNK, ch * MCHUNK)
                        ].rearrange("p (k m) -> p k m", k=ch),
                    )
                    xm_t.append(xj)

                # h2^T[f, m] accumulated over k-chunks; 4 PSUM banks live
                accs = [
                    ps_acc.tile(
                        [P, MCHUNK], F32, tag=f"acc{ft}", name=f"acc_{mc}_{ft}"
                    )
                    for ft in range(FT)
                ]
                yT = yT_pool.tile([P, FT, MCHUNK], DT, tag="yT")
                for j in range(NKCH):
                    for ft in range(FT):
                        for k in range(CHUNKS[j]):
                            nc.tensor.matmul(
                                accs[ft],
                                wfc_t[j][:, k, ts(ft, P)],
                                xm_t[j][:, k],
                                start=(j == 0 and k == 0),
                                stop=(j == NKCH - 1 and k == CHUNKS[j] - 1),
                            )
                for ft in range(FT):
                    # y^T = relu(h2^T + fc_b); bias is per-partition here
                    nc.scalar.activation(
                        yT[:, ft],
                        accs[ft],
                        mybir.ActivationFunctionType.Relu,
                        bias=fcb_sb[:, ds(ft, 1)],
                    )

                # logits: out[m, c] = sum_f y[m, f] lgw[c, f] + lgb[c]
                # all 4 msub accumulation groups share one PSUM tile (bank);
                # the start=True bank-wide has_written clear only resets
                # bits, not data, so disjoint column groups coexist.
                osb = osb_pool.tile([P, MSUB, CLS], F32, tag="osb")
                plg = ps_lg.tile([P, MSUB, CLS], F32, tag="lg", name=f"plg_{mc}")
                for msub in range(MSUB):
                    for ft in range(FT):
                        nc.tensor.matmul(
                            plg[:, msub],
                            yT[:, ft, ts(msub, P)],
                            lgwT_sb[:, ft],
                            start=(msub == 0 and ft == 0),
                            stop=False,
                            skip_group_check=True,
                        )
                nc.tensor.matmul(
                    plg.rearrange("p s c -> p (s c)"), ones_sb, lgb_rep,
                    start=False, stop=True, skip_group_check=True,
                )
                nc.vector.tensor_copy(osb, plg)

                nc.sync.dma_start(
                    out_d[ds(mc * MCHUNK, MCHUNK), :].rearrange(
                        "(s p) c -> p s c", p=P
                    ),
                    osb,
                )

    nc.compile()
    return nc


def kernel(**inputs) -> np.ndarray:
    global LAST_RESULT
    if "nc" not in _CACHE:
        _CACHE["nc"] = build_kernel()
    nc = _CACHE["nc"]

    x = np.ascontiguousarray(inputs["x"], dtype=np.float32)
    W = np.ascontiguousarray(inputs["W"], dtype=np.float32)
    fc_w = np.ascontiguousarray(inputs["fc_w"], dtype=np.float32)
    fc_b = np.ascontiguousarray(inputs["fc_b"], dtype=np.float32)
    lgw = np.ascontiguousarray(inputs["logits_w"], dtype=np.float32)
    lgb = np.ascontiguousarray(inputs["logits_b"], dtype=np.float32)

    # weight constant-fold + PE-friendly layouts
    wfc = W @ fc_w.T                                   # [N, FC]
    wfc_dev = np.ascontiguousarray(
        wfc.reshape(KT, P, FC).transpose(1, 0, 2).reshape(P, KT * FC).astype(NPDT)
    )
    cblob = np.zeros((P, 2 * FT * CLS + P), dtype=NPDT)
    cblob[:, : FT * CLS] = (
        lgw.T.astype(NPDT).reshape(FT, P, CLS).transpose(1, 0, 2).reshape(P, FT * CLS)
    )
    cblob[0, FT * CLS : 2 * FT * CLS] = np.tile(lgb.astype(NPDT), MSUB)
    cblob[0, 2 * FT * CLS :] = NPDT(1.0)

    in_maps = []
    for i in range(NCORES):
        xs = x[i * BS : (i + 1) * BS].astype(NPDT)     # [2048, 3072]
        # xTa rows (mc, p=k%128), cols (kt, m) : x^T tiles per m-chunk
        xta = np.ascontiguousarray(
            xs.T.reshape(KT, P, NMC, MCHUNK)
            .transpose(2, 1, 0, 3)
            .reshape(NMC * P, KT * MCHUNK)
        )
        in_maps.append(
            {
                "xTa": xta,
                "wfc": wfc_dev,
                "cblob": cblob,
                "fc_b": fc_b,
            }
        )

    res = run_bass_kernel_spmd(
        nc,
        in_maps,
        core_ids=list(range(NCORES)),
        trace=bool(int(os.environ.get("KERNEL_TRACE", "0"))),
    )
    LAST_RESULT = res
    out = np.concatenate([r_["out"] for r_ in res.results], axis=0)
    return out
